# revision 58
# baseline (speedup 1.0000x reference)
"""Multi-head attention Bass kernel for Trainium2, sharded over 8 NeuronCores.

Sharding: core c handles batch b = c//4 and head-group g = c%4 (4 of 16 heads,
i.e. a 256-wide slice of the QKV projection output).  Each core computes its
heads' attention and a partial output projection (contribution of its 256
ctx columns to the full [S, D] output).  The host sums the 4 partials per
batch and adds the output bias.

Device-side layout choices:
  - activations shipped pre-transposed: xT = x.T  [D, S] so the contraction
    dim (D) lands on SBUF partitions without any on-device transpose.
  - scores are computed transposed (scoresT[sk, sq]) so the attention weights
    leave softmax with sk on partitions — the contraction layout attn@V needs.
  - softmax denominator comes free from a ones-column appended to V
    (ctx psum row 64 = sum_sk attn);  no max-subtraction (scores bounded).
  - masking is a multiply by a 0/1 bf16 keep-mask after exp.

v2 pipeline structure (vs the original two-phase kernel):
  - projections run "n-outer": each 512-wide sequence chunk of K/V/Q is
    produced (and its psum evacuated) before the next, so attention for the
    first sq chunk starts ~25us in and the Scalar exp stream — the true
    engine floor for this shape — runs nearly the whole kernel.
  - the two heads of a pair write their score chunks into the two banks of
    a single [128,1024] psum tile: the K=64 matmuls auto-derive row-tile
    positions (0,0)/(64,0) and can run concurrently on the two halves of
    the PE array, and the softmax exp covers both heads in ONE N=1024
    activation (amortizing the ~352-cycle ACT ramp).
  - psum budget (8 banks): score tag 2x[128,1024] (4) + ctx tag 2x[65,512]
    as [128,512] (2) + outproj tag 2x[128,512] (2).  Projections flow
    through the score tag's slots before stage B claims them.
"""

import numpy as np
import ml_dtypes

import concourse.bass as bass
import concourse.mybir as mybir
import concourse.tile as tile
from concourse import bacc, library_config
from concourse.bass_utils import run_bass_kernel_spmd

# Problem shapes (hardcoded per contest rules).
B, S, D, H, DH = 2, 2048, 1024, 16, 64
NCORES = 8
NH = 4            # heads per core
DQ = NH * DH      # 256: per-core q/k/v width
P = 128

F32 = mybir.dt.float32
BF16 = mybir.dt.bfloat16
NP_BF16 = ml_dtypes.bfloat16

SQC = 512         # sq chunk for matmuls (one psum bank of fp32)
PN = 512          # projection free chunk


def build_nc(s=S, d=D):
    """Build the per-core Bass program (same NEFF on all 8 cores)."""
    ko = d // P             # 8 contraction chunks for projections
    mq = DQ // P            # 2 partition chunks of the per-core head width
    sk_n = s // P           # 16 key chunks
    nsq = s // SQC          # 4 query chunks
    npn = s // PN           # 4 projection chunks

    nc = bacc.Bacc("TRN2", debug=False)

    xq_t = nc.declare_dram_parameter("xq", [d, s], BF16, isOutput=False)
    xk_t = nc.declare_dram_parameter("xk", [d, s], BF16, isOutput=False)
    xv_t = nc.declare_dram_parameter("xv", [d, s], BF16, isOutput=False)
    wq_t = nc.declare_dram_parameter("wq", [d, DQ], BF16, isOutput=False)
    wk_t = nc.declare_dram_parameter("wk", [d, DQ], BF16, isOutput=False)
    wv_t = nc.declare_dram_parameter("wv", [d, DQ], BF16, isOutput=False)
    wo_t = nc.declare_dram_parameter("wo", [DQ, d], BF16, isOutput=False)
    bq_t = nc.declare_dram_parameter("bq", [P, mq], F32, isOutput=False)
    bk_t = nc.declare_dram_parameter("bk", [P, mq], F32, isOutput=False)
    bv_t = nc.declare_dram_parameter("bv", [P, DQ], F32, isOutput=False)
    keep_t = nc.declare_dram_parameter("keep", [s, s], BF16, isOutput=False)
    out_t = nc.declare_dram_parameter("out", [d, s], BF16, isOutput=True)

    AF = mybir.ActivationFunctionType
    OP = mybir.AluOpType

    with tile.TileContext(nc) as tc:
        with (
            tc.tile_pool(name="const", bufs=1) as const,
            tc.tile_pool(name="xs", bufs=4) as xs,
            tc.tile_pool(name="attn", bufs=3) as attnp,
            tc.tile_pool(name="sc", bufs=2) as scp,
            tc.tile_pool(name="outp", bufs=2) as outp,
            tc.tile_pool(name="ps", bufs=2, space="PSUM") as psp,
        ):
            # ---- persistent SBUF tensors ----
            wq_sb = const.tile([P, ko, DQ], BF16, tag="wq")
            wk_sb = const.tile([P, ko, DQ], BF16, tag="wk")
            wv_sb = const.tile([P, ko, DQ], BF16, tag="wv")
            wo_sb = const.tile([P, mq, d], BF16, tag="wo")
            bq_sb = const.tile([P, mq], F32, tag="bq")
            bk_sb = const.tile([P, mq], F32, tag="bk")
            bv_sb = const.tile([P, DQ], F32, tag="bv")
            qT_sb = const.tile([P, mq, s], BF16, tag="qT")
            kT_sb = const.tile([P, mq, s], BF16, tag="kT")
            v_sb = const.tile([P, sk_n, NH * 65], BF16, tag="v")
            keep_sb = const.tile([P, sk_n, s], BF16, tag="keep")
            ctxT_sb = const.tile([P, mq, s], BF16, tag="ctxT")
            xq_sb = const.tile([P, ko, s], BF16, tag="xq")
            xk_sb = const.tile([P, ko, s], BF16, tag="xk")

            # DMA queue order is deadline-driven: the first attention unit
            # needs wq + xq chunk 0 (sync) and wk + xk chunk 0 (scalar)
            # before anything else; wo isn't read until the first
            # out-projection ~60us in, so it goes last.  xq and xk are
            # prefetched whole, tile-by-tile in n-major order, so projection
            # matmuls never sit at the head of the in-order PE queue waiting
            # on a DMA.
            # The Scalar engine runs the exp stream: its queue carries ONLY
            # the DMAs needed before the first exp (wk, xk chunk 0, keep 0,
            # ~7us) — everything queued there would delay every exp behind
            # it in the in-order instruction stream.  The rest rides the
            # Sync queue (no compute behind it) in deadline order, and the
            # late keeps ride gpsimd after the xv waves.
            def keepd(eng, c):
                eng.dma_start(keep_sb[:, c, :], keep_t[c * P:(c + 1) * P, :])

            nc.sync.dma_start(wq_sb, wq_t[:].rearrange("(ko p) m -> p ko m", p=P))
            nc.scalar.dma_start(wk_sb, wk_t[:].rearrange("(ko p) m -> p ko m", p=P))
            for kk in range(ko):
                nc.sync.dma_start(
                    xq_sb[:, kk, 0:PN], xq_t[kk * P:(kk + 1) * P, 0:PN])
                nc.scalar.dma_start(
                    xk_sb[:, kk, 0:PN], xk_t[kk * P:(kk + 1) * P, 0:PN])
            nc.scalar.dma_start(keep_sb[:, 0, :], keep_t[0:P, :])
            for c in (1, 2, 3):
                keepd(nc.sync, c)
            nc.sync.dma_start(wv_sb, wv_t[:].rearrange("(ko p) m -> p ko m", p=P))
            nc.sync.dma_start(bq_sb, bq_t[:])
            nc.sync.dma_start(bk_sb, bk_t[:])
            nc.sync.dma_start(bv_sb, bv_t[:])
            for kk in range(ko):
                nc.sync.dma_start(
                    xk_sb[:, kk, PN:2 * PN], xk_t[kk * P:(kk + 1) * P, PN:2 * PN])
            keepd(nc.sync, 4)
            keepd(nc.sync, 5)
            for kk in range(ko):
                nc.sync.dma_start(
                    xk_sb[:, kk, 2 * PN:3 * PN],
                    xk_t[kk * P:(kk + 1) * P, 2 * PN:3 * PN])
            keepd(nc.sync, 6)
            keepd(nc.sync, 7)
            for kk in range(ko):
                nc.sync.dma_start(
                    xk_sb[:, kk, 3 * PN:4 * PN],
                    xk_t[kk * P:(kk + 1) * P, 3 * PN:4 * PN])
            keepd(nc.sync, 8)
            keepd(nc.sync, 9)
            for n in range(1, npn):
                for kk in range(ko):
                    nc.sync.dma_start(
                        xq_sb[:, kk, n * PN:(n + 1) * PN],
                        xq_t[kk * P:(kk + 1) * P, n * PN:(n + 1) * PN])
            nc.sync.dma_start(wo_sb, wo_t[:].rearrange("(mq p) n -> p mq n", p=P))
            for c in range(10, sk_n):
                keepd(nc.gpsimd, c)

            # ones column per head in the V tile (softmax denominator trick)
            v_strided = v_sb[:].rearrange("p s (h c) -> p s h c", h=NH)
            nc.vector.memset(v_strided[:, :, :, 64:65], 1.0)

            bv_h = bv_sb[:].rearrange("p (h c) -> p h c", h=NH)

            def sptile(name):
                # [128, 1024] fp32: two psum banks = two independent
                # accumulation zero-regions (cols 0:512 / 512:1024).
                return psp.tile([P, 2 * SQC], F32, name=name, tag="s", bufs=2)

            def proj_half(w_sb, x_sb, b_sb, dst_sb, n, m):
                """Half a projection chunk: output rows m*128..m*128+127 for
                sequence chunk n.  Pair p of the attention only reads the
                m=p half of Q/K, so the m=1 halves can weave in later."""
                ps = sptile("phps")
                for kk in range(ko):
                    nc.tensor.matmul(
                        ps[:, 0:PN],
                        w_sb[:, kk, m * P:(m + 1) * P],
                        x_sb[:, kk, n * PN:(n + 1) * PN],
                        start=(kk == 0),
                        stop=(kk == ko - 1),
                    )
                nc.vector.tensor_scalar_add(
                    dst_sb[:, m, n * PN:(n + 1) * PN],
                    ps[:, 0:PN],
                    b_sb[:, m:m + 1],
                )

            def proj_k_half(n, m):
                proj_half(wk_sb, xk_sb, bk_sb, kT_sb, n, m)

            def proj_q_half(n, m):
                proj_half(wq_sb, xq_sb, bq_sb, qT_sb, n, m)

            def gen_q_chunk_spread(n):
                """Q projection chunk as a generator: one matmul per step,
                on the out-proj psum slot, so mid-stream Q projection never
                bursts or steals a score-tile slot."""
                for m in range(mq):
                    ps = psp.tile([P, SQC], F32, name="qsp", tag="o", bufs=1)
                    for kk in range(ko):
                        nc.tensor.matmul(
                            ps,
                            wq_sb[:, kk, m * P:(m + 1) * P],
                            xq_sb[:, kk, n * PN:(n + 1) * PN],
                            start=(kk == 0),
                            stop=(kk == ko - 1),
                        )
                        yield
                    nc.vector.tensor_scalar_add(
                        qT_sb[:, m, n * PN:(n + 1) * PN], ps,
                        bq_sb[:, m:m + 1])

            def gen_out_proj_spread(sqc):
                """Out projection for chunk sqc, one do-slice per step."""
                sq0 = sqc * SQC
                for do in range(ko):
                    ps = psp.tile([P, SQC], F32, name="ops", tag="o", bufs=1)
                    for kkm in range(mq):
                        nc.tensor.matmul(
                            ps,
                            wo_sb[:, kkm, do * P:(do + 1) * P],
                            ctxT_sb[:, kkm, sq0:sq0 + SQC],
                            start=(kkm == 0),
                            stop=(kkm == mq - 1),
                        )
                    ot = outp.tile([P, SQC], BF16, tag="ot")
                    nc.vector.tensor_copy(ot, ps)
                    nc.sync.dma_start(
                        out_t[do * P:(do + 1) * P, sq0:sq0 + SQC], ot)
                    yield

            def proj_v_wave(w):
                """V projection for sv chunks 2w, 2w+1: one score-tag psum
                tile, one sv accumulation group per bank (so it can weave
                into the attention stream's slot rotation without touching
                the live ctx accumulators)."""
                ps = sptile("vps")
                for kk in range(ko):
                    t = xs.tile([P, 2 * P], BF16, tag="xv")
                    nc.gpsimd.dma_start(
                        t, xv_t[kk * P:(kk + 1) * P, w * 2 * P:(w + 1) * 2 * P])
                    for g in range(2):
                        nc.tensor.matmul(
                            ps[:, g * SQC:g * SQC + DQ],
                            t[:, g * P:(g + 1) * P],
                            wv_sb[:, kk, :],
                            start=(kk == 0),
                            stop=(kk == ko - 1),
                        )
                for g in range(2):
                    sv = 2 * w + g
                    nc.vector.tensor_tensor(
                        v_strided[:, sv, :, 0:64],
                        ps[:, g * SQC:g * SQC + DQ].rearrange(
                            "p (h c) -> p h c", h=NH),
                        bv_h,
                        OP.add,
                    )

            def normalize(cps, h, sq0):
                """ctx[0:64] /= den[64]; write into ctxT_sb (repacked).
                The FIRST op copies the whole ctx psum slice to SBUF so the
                psum slot frees ~1us after the last ctx matmul instead of
                being held through the whole reciprocal chain (that release
                gates the next pair's ctx matmuls at every pair boundary).
                HW quirk: custom-DVE / gpsimd ops only work at base partition
                0, so the den row is shifted there via an SBUF-SBUF DMA."""
                hb, hm = (h % 2) * 64, h // 2
                craw = scp.tile([65, SQC], F32, tag="craw")
                nc.vector.tensor_copy(craw, cps)
                den0 = scp.tile([1, SQC], F32, tag="den0")
                nc.gpsimd.dma_start(den0, craw[64:65, :])
                nc.vector.reciprocal_approx_fast(out=den0, in_=den0)
                scl = scp.tile([64, SQC], F32, tag="scl")
                nc.gpsimd.partition_broadcast(scl, den0[0:1, :])
                if hb == 0:
                    nc.vector.tensor_tensor(
                        ctxT_sb[0:64, hm, sq0:sq0 + SQC],
                        craw[0:64, :], scl, OP.mult)
                else:
                    cn = scp.tile([64, SQC], BF16, tag="cn")
                    nc.vector.tensor_tensor(cn, craw[0:64, :], scl, OP.mult)
                    nc.sync.dma_start(ctxT_sb[64:128, hm, sq0:sq0 + SQC], cn)

            def out_proj(sqc):
                sq0 = sqc * SQC
                for do in range(ko):
                    ps = psp.tile([P, SQC], F32, name="ops", tag="o", bufs=1)
                    for kkm in range(mq):
                        nc.tensor.matmul(
                            ps,
                            wo_sb[:, kkm, do * P:(do + 1) * P],
                            ctxT_sb[:, kkm, sq0:sq0 + SQC],
                            start=(kkm == 0),
                            stop=(kkm == mq - 1),
                        )
                    ot = outp.tile([P, SQC], BF16, tag="ot")
                    nc.vector.tensor_copy(ot, ps)
                    nc.sync.dma_start(
                        out_t[do * P:(do + 1) * P, sq0:sq0 + SQC], ot)

            # ---- stage B: one flat software pipeline over all units ----
            # Unit u = (sqc, pair, sk).  The ctx matmuls for unit u are
            # emitted after the score matmuls for unit u+2, so the in-order
            # PE queue never has a dependency-blocked ctx at its head: the
            # next scores (which only need a psum slot freed two exps ago)
            # and stage-A-style filler (Q proj, out proj) keep it streaming.
            units = [(sqc, pair, sk)
                     for sqc in range(nsq)
                     for pair in range(NH // 2)
                     for sk in range(sk_n)]
            cps_of = {}
            pend = []

            def drain_one():
                at, (sqc, pair, sk) = pend.pop(0)
                cpss = cps_of[(sqc, pair)]
                for i, h in enumerate((2 * pair, 2 * pair + 1)):
                    nc.tensor.matmul(
                        cpss[i],
                        v_sb[:, sk, h * 65:(h + 1) * 65],
                        at[:, i * SQC:(i + 1) * SQC],
                        start=(sk == 0),
                        stop=(sk == sk_n - 1),
                    )
                if sk == sk_n - 1:
                    sq0 = sqc * SQC
                    for i, h in enumerate((2 * pair, 2 * pair + 1)):
                        normalize(cpss[i], h, sq0)
                    del cps_of[(sqc, pair)]

            def emit_unit(u):
                sqc, pair, sk = u
                sq0 = sqc * SQC
                if sk == 0:
                    cps_of[(sqc, pair)] = [
                        psp.tile([P, SQC], F32, name=f"cps{i}", tag="c",
                                 bufs=3)[:65, :]
                        for i in range(2)]
                sps = sptile("sps")
                # two heads on the two halves of the PE array, outputs to
                # the tile's two psum banks
                nc.tensor.matmul(
                    sps[:, 0:SQC],
                    kT_sb[0:64, pair, sk * P:(sk + 1) * P],
                    qT_sb[0:64, pair, sq0:sq0 + SQC],
                    start=True, stop=True,
                )
                nc.tensor.matmul(
                    sps[:, SQC:2 * SQC],
                    kT_sb[64:128, pair, sk * P:(sk + 1) * P],
                    qT_sb[64:128, pair, sq0:sq0 + SQC],
                    start=True, stop=True,
                )
                if len(pend) >= 3:
                    drain_one()
                at = attnp.tile([P, 2 * SQC], BF16, tag="at", bufs=5)
                nc.scalar.activation(at, sps, AF.Exp, scale=0.125)
                for i in range(2):
                    nc.vector.tensor_tensor(
                        at[:, i * SQC:(i + 1) * SQC],
                        at[:, i * SQC:(i + 1) * SQC],
                        keep_sb[:, sk, sq0:sq0 + SQC],
                        OP.mult,
                    )
                pend.append((at, u))

            def out_proj_tail(sqc):
                """Final out projection: two do-slices per score-tag psum
                tile (the attention stream is done, so the slots are free
                and the tail runs wide instead of spread)."""
                sq0 = sqc * SQC
                for dd in range(ko // 2):
                    ps = sptile("otail")
                    for g in range(2):
                        do = 2 * dd + g
                        for kkm in range(mq):
                            nc.tensor.matmul(
                                ps[:, g * SQC:(g + 1) * SQC],
                                wo_sb[:, kkm, do * P:(do + 1) * P],
                                ctxT_sb[:, kkm, sq0:sq0 + SQC],
                                start=(kkm == 0),
                                stop=(kkm == mq - 1),
                            )
                    for g in range(2):
                        do = 2 * dd + g
                        ot = outp.tile([P, SQC], BF16, tag="ot")
                        nc.vector.tensor_copy(ot, ps[:, g * SQC:(g + 1) * SQC])
                        eng = (nc.sync, nc.gpsimd)[(2 * dd + g) % 2]
                        eng.dma_start(
                            out_t[do * P:(do + 1) * P, sq0:sq0 + SQC], ot)

            # ---- emission ----
            # A short burst of garbage matmuls (results never read) keeps
            # the PE busy from t=0 so the HAM clock-gate releases to 2.4GHz
            # before the real projection stream arrives.
            wu = psp.tile([P, SQC], F32, name="wu", tag="o", bufs=1)
            for i in range(32):
                nc.tensor.matmul(wu, ctxT_sb[:, 0, 0:128],
                                 ctxT_sb[:, 1, 0:SQC],
                                 start=(i == 0), stop=(i == 31))

            # Minimal prologue (K chunk 0 / Q chunk 0 m=0 halves, V waves
            # 0-1) so the first attention units are runnable ~10us in; the
            # REST of the K/V/Q projection weaves into the first attention
            # units as always-ready PE work (the in-order PE queue
            # guarantees each chunk executes before the attention unit that
            # consumes it).
            def keep_dma(c):
                nc.gpsimd.dma_start(
                    keep_sb[:, c, :], keep_t[c * P:(c + 1) * P, :])

            proj_q_half(0, 0)
            proj_k_half(0, 0)

            # Minimal prologue: unit 0's scores only need Q0m0 + K0m0.  The
            # V waves feed ctx matmuls that drain 3 units behind, so they
            # ALL weave; the gpsimd ucode library (16us queue occupancy,
            # only needed by the first normalize ~unit 18) loads after the
            # last xv DMA wave so it never blocks the value stream.
            stage_a = {
                0: lambda: proj_v_wave(0),
                1: lambda: proj_v_wave(1),
                2: lambda: proj_k_half(1, 0),
                3: lambda: proj_v_wave(2),
                4: lambda: proj_v_wave(3),
                5: lambda: proj_k_half(2, 0),
                6: lambda: proj_v_wave(4),
                7: lambda: proj_q_half(0, 1),
                8: lambda: proj_v_wave(5),
                9: lambda: proj_k_half(3, 0),
                10: lambda: proj_v_wave(6),
                11: lambda: proj_k_half(0, 1),
                12: lambda: proj_v_wave(7),
                13: lambda: nc.gpsimd.load_library(library_config.attn),
                14: lambda: proj_k_half(1, 1),
                15: lambda: proj_k_half(2, 1),
                16: lambda: proj_k_half(3, 1),
            }

            fillers = []
            for idx, u in enumerate(units):
                sqc, pair, sk = u
                emit_unit(u)
                if idx in stage_a:
                    stage_a[idx]()
                # mid-block filler, spread one matmul per unit: Q proj for
                # the next chunk, out proj for the previous one
                if pair == 0 and sk == 2 and sqc + 1 < nsq:
                    fillers.append(gen_q_chunk_spread(sqc + 1))
                if pair == 1 and sk == 6 and sqc >= 1:
                    fillers.append(gen_out_proj_spread(sqc - 1))
                if fillers:
                    try:
                        next(fillers[0])
                    except StopIteration:
                        fillers.pop(0)
            while pend:
                drain_one()
            for g in fillers:
                for _ in g:
                    pass
            # keep the PE busy through the final normalize chain so the
            # HAM clock-gate stays released for the tail out-projection
            wu2 = psp.tile([P, SQC], F32, name="wu2", tag="o", bufs=1)
            for i in range(24):
                nc.tensor.matmul(wu2, ctxT_sb[:, 0, 0:128],
                                 ctxT_sb[:, 1, 0:SQC],
                                 start=(i == 0), stop=(i == 23))
            out_proj_tail(nsq - 1)
    nc.compile()
    return nc


_NC_CACHE = {}


def _get_nc(s=S, d=D):
    key = (s, d, SQC)
    if key not in _NC_CACHE:
        _NC_CACHE[key] = build_nc(s, d)
    return _NC_CACHE[key]


def make_in_maps(query, key, value, mask, Wq, bq, Wk, bk, Wv, bv, Wo, bo,
                 s=S, d=D):
    """Build the 8 per-core input maps (host-side shard + layout prep)."""
    nb = query.shape[0]
    per_b = []
    for b in range(nb):
        xqT = np.ascontiguousarray(query[b].T).astype(NP_BF16)
        xkT = np.ascontiguousarray(key[b].T).astype(NP_BF16)
        xvT = np.ascontiguousarray(value[b].T).astype(NP_BF16)
        keepT = np.ascontiguousarray((~mask[b, 0]).T).astype(NP_BF16)
        per_b.append((xqT, xkT, xvT, keepT))
    per_g = []
    for g in range(4):
        sl = slice(g * DQ, (g + 1) * DQ)
        per_g.append((
            np.ascontiguousarray(Wq[sl].T).astype(NP_BF16),
            np.ascontiguousarray(Wk[sl].T).astype(NP_BF16),
            np.ascontiguousarray(Wv[sl].T).astype(NP_BF16),
            np.ascontiguousarray(Wo[:, sl].T).astype(NP_BF16),
            np.ascontiguousarray(bq[sl].reshape(DQ // P, P).T).astype(np.float32),
            np.ascontiguousarray(bk[sl].reshape(DQ // P, P).T).astype(np.float32),
            np.ascontiguousarray(np.broadcast_to(bv[sl], (P, DQ))).astype(np.float32),
        ))
    in_maps = []
    for c in range(NCORES):
        b, g = c // 4, c % 4
        xqT, xkT, xvT, keepT = per_b[b % nb]
        wqT, wkT, wvT, woT, bq2, bk2, bvr = per_g[g]
        in_maps.append({
            "xq": xqT, "xk": xkT, "xv": xvT,
            "wq": wqT, "wk": wkT, "wv": wvT, "wo": woT,
            "bq": bq2, "bk": bk2, "bv": bvr,
            "keep": keepT,
        })
    return in_maps


def gather_output(results, bo, nb=B, s=S, d=D):
    out = np.empty((nb, s, d), np.float32)
    for b in range(nb):
        acc = results[4 * b]["out"].astype(np.float32)
        for g in range(1, 4):
            acc += results[4 * b + g]["out"].astype(np.float32)
        out[b] = acc.T
    out += bo.astype(np.float32)
    return out


def run_on_cores(in_maps, trace=False, **kw):
    nc = _get_nc()
    return run_bass_kernel_spmd(nc, in_maps, list(range(NCORES)), trace=trace, **kw)


def kernel(query, key, value, mask, Wq, bq, Wk, bk, Wv, bv, Wo, bo):
    in_maps = make_in_maps(query, key, value, mask,
                           Wq, bq, Wk, bk, Wv, bv, Wo, bo)
    res = run_on_cores(in_maps, trace=False)
    return gather_output(res.results, bo)


# revision 60
# speedup vs baseline: 1.1797x; 1.1797x over previous
"""Multi-head attention Bass kernel for Trainium2, sharded over 8 NeuronCores.

Sharding: core c handles batch b = c//4 and head-group g = c%4 (4 of 16 heads,
i.e. a 256-wide slice of the QKV projection output).  Each core computes its
heads' attention and a partial output projection (contribution of its 256
ctx columns to the full [S, D] output).  The host sums the 4 partials per
batch and adds the output bias.

Device-side layout choices:
  - activations shipped pre-transposed: xT = x.T  [D, S] so the contraction
    dim (D) lands on SBUF partitions without any on-device transpose.
  - scores are computed transposed (scoresT[sk, sq]) so the attention weights
    leave softmax with sk on partitions — the contraction layout attn@V needs.
  - softmax denominator comes free from a ones-column appended to V
    (ctx psum row 64 = sum_sk attn);  no max-subtraction (scores bounded).
  - masking is a multiply by a 0/1 bf16 keep-mask after exp.

v2 pipeline structure (vs the original two-phase kernel):
  - projections run "n-outer": each 512-wide sequence chunk of K/V/Q is
    produced (and its psum evacuated) before the next, so attention for the
    first sq chunk starts ~25us in and the Scalar exp stream — the true
    engine floor for this shape — runs nearly the whole kernel.
  - the two heads of a pair write their score chunks into the two banks of
    a single [128,1024] psum tile: the K=64 matmuls auto-derive row-tile
    positions (0,0)/(64,0) and can run concurrently on the two halves of
    the PE array, and the softmax exp covers both heads in ONE N=1024
    activation (amortizing the ~352-cycle ACT ramp).
  - psum budget (8 banks): score tag 2x[128,1024] (4) + ctx tag 2x[65,512]
    as [128,512] (2) + outproj tag 2x[128,512] (2).  Projections flow
    through the score tag's slots before stage B claims them.
"""

import numpy as np
import ml_dtypes

import concourse.bass as bass
import concourse.mybir as mybir
import concourse.tile as tile
from concourse import bacc, library_config
from concourse.bass_utils import run_bass_kernel_spmd

# Problem shapes (hardcoded per contest rules).
B, S, D, H, DH = 2, 2048, 1024, 16, 64
NCORES = 8
NH = 4            # heads per core
DQ = NH * DH      # 256: per-core q/k/v width
P = 128

F32 = mybir.dt.float32
BF16 = mybir.dt.bfloat16
NP_BF16 = ml_dtypes.bfloat16

SQC = 512         # sq chunk for matmuls (one psum bank of fp32)
PN = 512          # projection free chunk


def build_nc(s=S, d=D):
    """Build the per-core Bass program (same NEFF on all 8 cores)."""
    ko = d // P             # 8 contraction chunks for projections
    mq = DQ // P            # 2 partition chunks of the per-core head width
    sk_n = s // P           # 16 key chunks
    nsq = s // SQC          # 4 query chunks
    npn = s // PN           # 4 projection chunks

    nc = bacc.Bacc("TRN2", debug=False)

    xq_t = nc.declare_dram_parameter("xq", [d, s], BF16, isOutput=False)
    xk_t = nc.declare_dram_parameter("xk", [d, s], BF16, isOutput=False)
    xv_t = nc.declare_dram_parameter("xv", [d, s], BF16, isOutput=False)
    wq_t = nc.declare_dram_parameter("wq", [d, DQ], BF16, isOutput=False)
    wk_t = nc.declare_dram_parameter("wk", [d, DQ], BF16, isOutput=False)
    wv_t = nc.declare_dram_parameter("wv", [d, DQ], BF16, isOutput=False)
    wo_t = nc.declare_dram_parameter("wo", [DQ, d], BF16, isOutput=False)
    bq_t = nc.declare_dram_parameter("bq", [P, mq], F32, isOutput=False)
    bk_t = nc.declare_dram_parameter("bk", [P, mq], F32, isOutput=False)
    bv_t = nc.declare_dram_parameter("bv", [P, DQ], F32, isOutput=False)
    keep_t = nc.declare_dram_parameter("keep", [s, s], BF16, isOutput=False)
    out_t = nc.declare_dram_parameter("out", [d, s], BF16, isOutput=True)

    AF = mybir.ActivationFunctionType
    OP = mybir.AluOpType

    with tile.TileContext(nc) as tc:
        nc.gpsimd.load_library(library_config.attn)
        with (
            tc.tile_pool(name="const", bufs=1) as const,
            tc.tile_pool(name="xs", bufs=4) as xs,
            tc.tile_pool(name="attn", bufs=3) as attnp,
            tc.tile_pool(name="sc", bufs=2) as scp,
            tc.tile_pool(name="outp", bufs=2) as outp,
            tc.tile_pool(name="ps", bufs=2, space="PSUM") as psp,
        ):
            # ---- persistent SBUF tensors ----
            wq_sb = const.tile([P, ko, DQ], BF16, tag="wq")
            wk_sb = const.tile([P, ko, DQ], BF16, tag="wk")
            wv_sb = const.tile([P, ko, DQ], BF16, tag="wv")
            wo_sb = const.tile([P, mq, d], BF16, tag="wo")
            bq_sb = const.tile([P, mq], F32, tag="bq")
            bk_sb = const.tile([P, mq], F32, tag="bk")
            bv_sb = const.tile([P, DQ], F32, tag="bv")
            qT_sb = const.tile([P, mq, s], BF16, tag="qT")
            kT_sb = const.tile([P, mq, s], BF16, tag="kT")
            v_sb = const.tile([P, sk_n, NH * 65], BF16, tag="v")
            keep_sb = const.tile([P, sk_n, s], BF16, tag="keep")
            ctxT_sb = const.tile([P, mq, s], BF16, tag="ctxT")
            xq_sb = const.tile([P, ko, s], BF16, tag="xq")
            xk_sb = const.tile([P, ko, s], BF16, tag="xk")

            # DMA queue order is deadline-driven: the first attention unit
            # needs wq + xq chunk 0 (sync) and wk + xk chunk 0 (scalar)
            # before anything else; wo isn't read until the first
            # out-projection ~60us in, so it goes last.  xq and xk are
            # prefetched whole, tile-by-tile in n-major order, so projection
            # matmuls never sit at the head of the in-order PE queue waiting
            # on a DMA.
            nc.sync.dma_start(wq_sb, wq_t[:].rearrange("(ko p) m -> p ko m", p=P))
            nc.scalar.dma_start(wk_sb, wk_t[:].rearrange("(ko p) m -> p ko m", p=P))
            for kk in range(ko):
                nc.sync.dma_start(
                    xq_sb[:, kk, 0:PN], xq_t[kk * P:(kk + 1) * P, 0:PN])
                nc.scalar.dma_start(
                    xk_sb[:, kk, 0:PN], xk_t[kk * P:(kk + 1) * P, 0:PN])
            nc.sync.dma_start(keep_sb[:, 1, :], keep_t[P:2 * P, :])
            nc.sync.dma_start(wv_sb, wv_t[:].rearrange("(ko p) m -> p ko m", p=P))
            nc.sync.dma_start(bq_sb, bq_t[:])
            nc.sync.dma_start(bk_sb, bk_t[:])
            nc.sync.dma_start(bv_sb, bv_t[:])
            # Scalar carries NOTHING past xk chunk 0: every DMA there delays
            # the whole in-order exp stream behind it.  The remaining keeps
            # ride Sync ahead of the xq chunks (Q-gen reads those ~60us+);
            # xk chunks 1-3 ride gpsimd from weave positions (below).
            for c in range(2, 13):
                nc.sync.dma_start(
                    keep_sb[:, c, :], keep_t[c * P:(c + 1) * P, :])
            for kk in range(ko):
                nc.sync.dma_start(
                    xq_sb[:, kk, PN:2 * PN], xq_t[kk * P:(kk + 1) * P, PN:2 * PN])
            for c in range(13, sk_n):
                nc.sync.dma_start(
                    keep_sb[:, c, :], keep_t[c * P:(c + 1) * P, :])
            for kk in range(ko):
                nc.sync.dma_start(
                    xq_sb[:, kk, 2 * PN:3 * PN],
                    xq_t[kk * P:(kk + 1) * P, 2 * PN:3 * PN])
            nc.sync.dma_start(wo_sb, wo_t[:].rearrange("(mq p) n -> p mq n", p=P))
            for kk in range(ko):
                nc.sync.dma_start(
                    xq_sb[:, kk, 3 * PN:4 * PN],
                    xq_t[kk * P:(kk + 1) * P, 3 * PN:4 * PN])

            def xk_dma(n):
                for kk in range(ko):
                    nc.gpsimd.dma_start(
                        xk_sb[:, kk, n * PN:(n + 1) * PN],
                        xk_t[kk * P:(kk + 1) * P, n * PN:(n + 1) * PN])

            # ones column per head in the V tile (softmax denominator trick)
            v_strided = v_sb[:].rearrange("p s (h c) -> p s h c", h=NH)
            nc.vector.memset(v_strided[:, :, :, 64:65], 1.0)

            bv_h = bv_sb[:].rearrange("p (h c) -> p h c", h=NH)

            def sptile(name):
                # [128, 1024] fp32: two psum banks = two independent
                # accumulation zero-regions (cols 0:512 / 512:1024).
                return psp.tile([P, 2 * SQC], F32, name=name, tag="s", bufs=2)

            def proj_half(w_sb, x_sb, b_sb, dst_sb, n, m):
                """Half a projection chunk: output rows m*128..m*128+127 for
                sequence chunk n.  Pair p of the attention only reads the
                m=p half of Q/K, so the m=1 halves can weave in later."""
                ps = sptile("phps")
                for kk in range(ko):
                    nc.tensor.matmul(
                        ps[:, 0:PN],
                        w_sb[:, kk, m * P:(m + 1) * P],
                        x_sb[:, kk, n * PN:(n + 1) * PN],
                        start=(kk == 0),
                        stop=(kk == ko - 1),
                    )
                nc.vector.tensor_scalar_add(
                    dst_sb[:, m, n * PN:(n + 1) * PN],
                    ps[:, 0:PN],
                    b_sb[:, m:m + 1],
                )

            def proj_k_half(n, m):
                proj_half(wk_sb, xk_sb, bk_sb, kT_sb, n, m)

            def proj_q_half(n, m):
                proj_half(wq_sb, xq_sb, bq_sb, qT_sb, n, m)

            def gen_q_chunk_spread(n):
                """Q projection chunk as a generator: one matmul per step,
                on the out-proj psum slot, so mid-stream Q projection never
                bursts or steals a score-tile slot."""
                for m in range(mq):
                    ps = psp.tile([P, SQC], F32, name="qsp", tag="o", bufs=1)
                    for kk in range(ko):
                        nc.tensor.matmul(
                            ps,
                            wq_sb[:, kk, m * P:(m + 1) * P],
                            xq_sb[:, kk, n * PN:(n + 1) * PN],
                            start=(kk == 0),
                            stop=(kk == ko - 1),
                        )
                        yield
                    nc.vector.tensor_scalar_add(
                        qT_sb[:, m, n * PN:(n + 1) * PN], ps,
                        bq_sb[:, m:m + 1])

            def gen_out_proj_spread(sqc):
                """Out projection for chunk sqc, one do-slice per step."""
                sq0 = sqc * SQC
                for do in range(ko):
                    ps = psp.tile([P, SQC], F32, name="ops", tag="o", bufs=1)
                    for kkm in range(mq):
                        nc.tensor.matmul(
                            ps,
                            wo_sb[:, kkm, do * P:(do + 1) * P],
                            ctxT_sb[:, kkm, sq0:sq0 + SQC],
                            start=(kkm == 0),
                            stop=(kkm == mq - 1),
                        )
                    ot = outp.tile([P, SQC], BF16, tag="ot")
                    nc.vector.tensor_copy(ot, ps)
                    nc.sync.dma_start(
                        out_t[do * P:(do + 1) * P, sq0:sq0 + SQC], ot)
                    yield

            def proj_v_wave(w):
                """V projection for sv chunks 2w, 2w+1: one score-tag psum
                tile, one sv accumulation group per bank (so it can weave
                into the attention stream's slot rotation without touching
                the live ctx accumulators)."""
                ps = sptile("vps")
                for kk in range(ko):
                    t = xs.tile([P, 2 * P], BF16, tag="xv")
                    nc.gpsimd.dma_start(
                        t, xv_t[kk * P:(kk + 1) * P, w * 2 * P:(w + 1) * 2 * P])
                    for g in range(2):
                        nc.tensor.matmul(
                            ps[:, g * SQC:g * SQC + DQ],
                            t[:, g * P:(g + 1) * P],
                            wv_sb[:, kk, :],
                            start=(kk == 0),
                            stop=(kk == ko - 1),
                        )
                for g in range(2):
                    sv = 2 * w + g
                    nc.vector.tensor_tensor(
                        v_strided[:, sv, :, 0:64],
                        ps[:, g * SQC:g * SQC + DQ].rearrange(
                            "p (h c) -> p h c", h=NH),
                        bv_h,
                        OP.add,
                    )

            def normalize(cps, h, sq0):
                """ctx[0:64] /= den[64]; write into ctxT_sb (repacked).
                The FIRST op copies the whole ctx psum slice to SBUF so the
                psum slot frees ~1us after the last ctx matmul instead of
                being held through the whole reciprocal chain (that release
                gates the next pair's ctx matmuls at every pair boundary).
                HW quirk: custom-DVE / gpsimd ops only work at base partition
                0, so the den row is shifted there via an SBUF-SBUF DMA."""
                hb, hm = (h % 2) * 64, h // 2
                craw = scp.tile([65, SQC], F32, tag="craw")
                nc.vector.tensor_copy(craw, cps)
                den0 = scp.tile([1, SQC], F32, tag="den0")
                nc.gpsimd.dma_start(den0, craw[64:65, :])
                nc.vector.reciprocal_approx_fast(out=den0, in_=den0)
                scl = scp.tile([64, SQC], F32, tag="scl")
                nc.gpsimd.partition_broadcast(scl, den0[0:1, :])
                if hb == 0:
                    nc.vector.tensor_tensor(
                        ctxT_sb[0:64, hm, sq0:sq0 + SQC],
                        craw[0:64, :], scl, OP.mult)
                else:
                    cn = scp.tile([64, SQC], BF16, tag="cn")
                    nc.vector.tensor_tensor(cn, craw[0:64, :], scl, OP.mult)
                    nc.sync.dma_start(ctxT_sb[64:128, hm, sq0:sq0 + SQC], cn)

            def out_proj(sqc):
                sq0 = sqc * SQC
                for do in range(ko):
                    ps = psp.tile([P, SQC], F32, name="ops", tag="o", bufs=1)
                    for kkm in range(mq):
                        nc.tensor.matmul(
                            ps,
                            wo_sb[:, kkm, do * P:(do + 1) * P],
                            ctxT_sb[:, kkm, sq0:sq0 + SQC],
                            start=(kkm == 0),
                            stop=(kkm == mq - 1),
                        )
                    ot = outp.tile([P, SQC], BF16, tag="ot")
                    nc.vector.tensor_copy(ot, ps)
                    nc.sync.dma_start(
                        out_t[do * P:(do + 1) * P, sq0:sq0 + SQC], ot)

            # ---- stage B: one flat software pipeline over all units ----
            # Unit u = (sqc, pair, sk).  The ctx matmuls for unit u are
            # emitted after the score matmuls for unit u+2, so the in-order
            # PE queue never has a dependency-blocked ctx at its head: the
            # next scores (which only need a psum slot freed two exps ago)
            # and stage-A-style filler (Q proj, out proj) keep it streaming.
            units = [(sqc, pair, sk)
                     for sqc in range(nsq)
                     for pair in range(NH // 2)
                     for sk in range(sk_n)]
            cps_of = {}
            pend = []

            def drain_one():
                at, (sqc, pair, sk) = pend.pop(0)
                cpss = cps_of[(sqc, pair)]
                for i, h in enumerate((2 * pair, 2 * pair + 1)):
                    nc.tensor.matmul(
                        cpss[i],
                        v_sb[:, sk, h * 65:(h + 1) * 65],
                        at[:, i * SQC:(i + 1) * SQC],
                        start=(sk == 0),
                        stop=(sk == sk_n - 1),
                    )
                if sk == sk_n - 1:
                    sq0 = sqc * SQC
                    for i, h in enumerate((2 * pair, 2 * pair + 1)):
                        normalize(cpss[i], h, sq0)
                    del cps_of[(sqc, pair)]

            def emit_unit(u):
                sqc, pair, sk = u
                sq0 = sqc * SQC
                if sk == 0:
                    cps_of[(sqc, pair)] = [
                        psp.tile([P, SQC], F32, name=f"cps{i}", tag="c",
                                 bufs=3)[:65, :]
                        for i in range(2)]
                sps = sptile("sps")
                # two heads on the two halves of the PE array, outputs to
                # the tile's two psum banks
                nc.tensor.matmul(
                    sps[:, 0:SQC],
                    kT_sb[0:64, pair, sk * P:(sk + 1) * P],
                    qT_sb[0:64, pair, sq0:sq0 + SQC],
                    start=True, stop=True,
                )
                nc.tensor.matmul(
                    sps[:, SQC:2 * SQC],
                    kT_sb[64:128, pair, sk * P:(sk + 1) * P],
                    qT_sb[64:128, pair, sq0:sq0 + SQC],
                    start=True, stop=True,
                )
                if len(pend) >= 3:
                    drain_one()
                at = attnp.tile([P, 2 * SQC], BF16, tag="at", bufs=5)
                nc.scalar.activation(at, sps, AF.Exp, scale=0.125)
                for i in range(2):
                    nc.vector.tensor_tensor(
                        at[:, i * SQC:(i + 1) * SQC],
                        at[:, i * SQC:(i + 1) * SQC],
                        keep_sb[:, sk, sq0:sq0 + SQC],
                        OP.mult,
                    )
                pend.append((at, u))

            def out_proj_tail(sqc):
                """Final out projection: two do-slices per score-tag psum
                tile (the attention stream is done, so the slots are free
                and the tail runs wide instead of spread)."""
                sq0 = sqc * SQC
                for dd in range(ko // 2):
                    ps = sptile("otail")
                    for g in range(2):
                        do = 2 * dd + g
                        for kkm in range(mq):
                            nc.tensor.matmul(
                                ps[:, g * SQC:(g + 1) * SQC],
                                wo_sb[:, kkm, do * P:(do + 1) * P],
                                ctxT_sb[:, kkm, sq0:sq0 + SQC],
                                start=(kkm == 0),
                                stop=(kkm == mq - 1),
                            )
                    for g in range(2):
                        do = 2 * dd + g
                        ot = outp.tile([P, SQC], BF16, tag="ot")
                        nc.vector.tensor_copy(ot, ps[:, g * SQC:(g + 1) * SQC])
                        eng = (nc.sync, nc.gpsimd)[(2 * dd + g) % 2]
                        eng.dma_start(
                            out_t[do * P:(do + 1) * P, sq0:sq0 + SQC], ot)

            # ---- emission ----
            # A short burst of garbage matmuls (results never read) keeps
            # the PE busy from t=0 so the HAM clock-gate releases to 2.4GHz
            # before the real projection stream arrives.
            wu = psp.tile([P, SQC], F32, name="wu", tag="o", bufs=1)
            for i in range(32):
                nc.tensor.matmul(wu, ctxT_sb[:, 0, 0:128],
                                 ctxT_sb[:, 1, 0:SQC],
                                 start=(i == 0), stop=(i == 31))

            # Minimal prologue (K chunk 0 / Q chunk 0 m=0 halves, V waves
            # 0-1) so the first attention units are runnable ~10us in; the
            # REST of the K/V/Q projection weaves into the first attention
            # units as always-ready PE work (the in-order PE queue
            # guarantees each chunk executes before the attention unit that
            # consumes it).
            def keep_dma(c):
                nc.gpsimd.dma_start(
                    keep_sb[:, c, :], keep_t[c * P:(c + 1) * P, :])

            proj_k_half(0, 0)
            proj_v_wave(0)
            keep_dma(0)
            proj_q_half(0, 0)

            # V wave 1 (key chunks 2-3, first read at unit 2) moves out of
            # the prologue into the weave so the first score matmuls are not
            # queued behind its 16 matmuls in the in-order PE stream.
            def w_idx0():
                xk_dma(1)
                proj_v_wave(1)

            def w_idx1():
                xk_dma(2)
                proj_v_wave(2)

            def w_idx4():
                xk_dma(3)
                proj_q_half(0, 1)

            stage_a = {
                0: w_idx0,
                1: w_idx1,
                2: lambda: proj_k_half(1, 0),
                3: lambda: proj_v_wave(3),
                4: w_idx4,
                5: lambda: proj_v_wave(4),
                6: lambda: proj_k_half(2, 0),
                7: lambda: proj_k_half(0, 1),
                8: lambda: proj_v_wave(5),
                9: lambda: proj_v_wave(6),
                10: lambda: proj_k_half(3, 0),
                11: lambda: proj_k_half(1, 1),
                12: lambda: proj_v_wave(7),
                13: lambda: proj_k_half(2, 1),
                14: lambda: proj_k_half(3, 1),
            }

            fillers = []
            for idx, u in enumerate(units):
                sqc, pair, sk = u
                emit_unit(u)
                if idx in stage_a:
                    stage_a[idx]()
                # mid-block filler, spread one matmul per unit: Q proj for
                # the next chunk, out proj for the previous one
                if pair == 0 and sk == 2 and sqc + 1 < nsq:
                    fillers.append(gen_q_chunk_spread(sqc + 1))
                if pair == 1 and sk == 6 and sqc >= 1:
                    fillers.append(gen_out_proj_spread(sqc - 1))
                if fillers:
                    try:
                        next(fillers[0])
                    except StopIteration:
                        fillers.pop(0)
            while pend:
                drain_one()
            for g in fillers:
                for _ in g:
                    pass
            # keep the PE busy through the final normalize chain so the
            # HAM clock-gate stays released for the tail out-projection
            wu2 = psp.tile([P, SQC], F32, name="wu2", tag="o", bufs=1)
            for i in range(24):
                nc.tensor.matmul(wu2, ctxT_sb[:, 0, 0:128],
                                 ctxT_sb[:, 1, 0:SQC],
                                 start=(i == 0), stop=(i == 23))
            out_proj_tail(nsq - 1)
    nc.compile()
    return nc


_NC_CACHE = {}


def _get_nc(s=S, d=D):
    key = (s, d, SQC)
    if key not in _NC_CACHE:
        _NC_CACHE[key] = build_nc(s, d)
    return _NC_CACHE[key]


def make_in_maps(query, key, value, mask, Wq, bq, Wk, bk, Wv, bv, Wo, bo,
                 s=S, d=D):
    """Build the 8 per-core input maps (host-side shard + layout prep)."""
    nb = query.shape[0]
    per_b = []
    for b in range(nb):
        xqT = np.ascontiguousarray(query[b].T).astype(NP_BF16)
        xkT = np.ascontiguousarray(key[b].T).astype(NP_BF16)
        xvT = np.ascontiguousarray(value[b].T).astype(NP_BF16)
        keepT = np.ascontiguousarray((~mask[b, 0]).T).astype(NP_BF16)
        per_b.append((xqT, xkT, xvT, keepT))
    per_g = []
    for g in range(4):
        sl = slice(g * DQ, (g + 1) * DQ)
        per_g.append((
            np.ascontiguousarray(Wq[sl].T).astype(NP_BF16),
            np.ascontiguousarray(Wk[sl].T).astype(NP_BF16),
            np.ascontiguousarray(Wv[sl].T).astype(NP_BF16),
            np.ascontiguousarray(Wo[:, sl].T).astype(NP_BF16),
            np.ascontiguousarray(bq[sl].reshape(DQ // P, P).T).astype(np.float32),
            np.ascontiguousarray(bk[sl].reshape(DQ // P, P).T).astype(np.float32),
            np.ascontiguousarray(np.broadcast_to(bv[sl], (P, DQ))).astype(np.float32),
        ))
    in_maps = []
    for c in range(NCORES):
        b, g = c // 4, c % 4
        xqT, xkT, xvT, keepT = per_b[b % nb]
        wqT, wkT, wvT, woT, bq2, bk2, bvr = per_g[g]
        in_maps.append({
            "xq": xqT, "xk": xkT, "xv": xvT,
            "wq": wqT, "wk": wkT, "wv": wvT, "wo": woT,
            "bq": bq2, "bk": bk2, "bv": bvr,
            "keep": keepT,
        })
    return in_maps


def gather_output(results, bo, nb=B, s=S, d=D):
    out = np.empty((nb, s, d), np.float32)
    for b in range(nb):
        acc = results[4 * b]["out"].astype(np.float32)
        for g in range(1, 4):
            acc += results[4 * b + g]["out"].astype(np.float32)
        out[b] = acc.T
    out += bo.astype(np.float32)
    return out


def run_on_cores(in_maps, trace=False, **kw):
    nc = _get_nc()
    return run_bass_kernel_spmd(nc, in_maps, list(range(NCORES)), trace=trace, **kw)


def kernel(query, key, value, mask, Wq, bq, Wk, bk, Wv, bv, Wo, bo):
    in_maps = make_in_maps(query, key, value, mask,
                           Wq, bq, Wk, bk, Wv, bv, Wo, bo)
    res = run_on_cores(in_maps, trace=False)
    return gather_output(res.results, bo)
